# revision 4
# baseline (speedup 1.0000x reference)
"""Token-major hypergraph conv kernel for TRN2 (8 cores, SPMD).

Design:
  - phase1: per 128-node chunk, one bf16 matmul [x^T chunk | Wn|Wattn] ->
    tokens [snf(128) | exp(4) | pad] written to snf_hbm (node-major, 512B rows).
  - pass A: per-incidence non-transpose dma_gather of 512B tokens (runs of a
    segment live on one partition row); free-dim tensor_reduce over k ->
    token-major run sums; staging flushed DENSE to Uperm_hbm (HWDGE).
  - fixA: inverse-permutation gather -> dense U_hbm [25600, 256] (U128|D4).
  - ReduceScatter(U_hbm) -> per-rank [3200, 256]; EA' = (U/D + ef)/D with ef
    matmul'd on the fly; AllGather -> EAp_full [25600, 128] bf16.
  - pass B: same gather/reduce over node-sorted incidences -> yperm_hbm.
  - fixB: inverse-perm gather + *exp + bias -> y [12544, 128] f32.
"""
import os
import numpy as np
from dataclasses import dataclass

import bass_rust
import concourse.bass as bass
import concourse.mybir as mybir
import concourse.bacc as bacc
import concourse.tile as tile
from concourse.library_config import mlp as mlp_library
from concourse._compat import get_trn_type

F32 = mybir.dt.float32
BF16 = mybir.dt.bfloat16
I16 = mybir.dt.int16
AX = mybir.AxisListType
ALU = mybir.AluOpType
ACTF = mybir.ActivationFunctionType

N, E, D, H, C = 100000, 25000, 128, 4, 32
NC, NSH = 8, 12500
ET = 25600              # padded edges (200*128; /8 = 3200 = 25*128)
NT = 12544              # node tokens (98*128; 12500 real + dummy at 12500)
ESH = ET // NC          # 3200 edge rows per rank after RS
NSPLIT = 5              # collective pipeline slices
ESPL = ET // NSPLIT     # 5120 edges per slice
ESHS = ESPL // NC       # 640 rows per rank per slice
DUMMY_A = NSH           # pass A dummy gather target (zero token)
DUMMY_B = ET - 1        # pass B dummy gather target (EAp row == 0)

NI = 4096               # gather idxs per instruction
NBLK = NI // 128
FSEG = 16               # segments per staging flush
DEPTH_A = 6             # gather tile rotation depth
DEPTH_B = 8


# ===================== host planner =====================

def _runs(keys):
    if len(keys) == 0:
        return np.zeros(0, np.int64), np.zeros(0, np.int64), np.zeros(0, np.int64)
    change = np.flatnonzero(np.diff(keys)) + 1
    starts = np.concatenate([[0], change]).astype(np.int64)
    ends = np.concatenate([change, [len(keys)]]).astype(np.int64)
    return starts, ends - starts, keys[starts].astype(np.int64)


@dataclass
class PassPlan:
    gidx: list      # per core [slots] gather token ids
    pos: list       # per core [total_keys] staging row of each key
    ops: list       # common ("tr", chunk, blk0, k, n, sblk) / ("flush", nseg)
    nchunks: int
    slots: int
    nflush: int
    nstage_rows: int


def plan_pass(per_core_keys_vals, total_keys, dummy_vals, ni=NI):
    core_runs = []
    for keys, vals in per_core_keys_vals:
        starts, lens, kv = _runs(np.asarray(keys, np.int64))
        run_len = np.zeros(total_keys, np.int64)
        run_start = np.full(total_keys, -1, np.int64)
        run_len[kv] = lens
        run_start[kv] = starts
        order = np.argsort(-np.maximum(run_len, 1), kind="stable")
        core_runs.append((run_len[order], run_start[order], order))
    nseg = (total_keys + 127) // 128
    seg_k = np.zeros(nseg, np.int64)
    for s in range(nseg):
        for lens, _, _ in core_runs:
            seg_k[s] = max(seg_k[s], max(int(lens[s * 128]), 1))
    nblk = ni // 128
    assert seg_k.max() <= nblk, f"seg width {seg_k.max()} > {nblk}"
    ops = []
    seg_pos = []
    chunk = blk = 0
    fseg = 0
    for s in range(nseg):
        k = int(seg_k[s])
        n = min(128, total_keys - s * 128)
        if blk + k > nblk:
            chunk += 1
            blk = 0
        ops.append(("tr", chunk, blk, k, n, fseg))
        seg_pos.append((chunk, blk))
        blk += k
        fseg += 1
        if fseg == FSEG:
            ops.append(("flush", fseg))
            fseg = 0
    if fseg:
        ops.append(("flush", fseg))
    nchunks = chunk + 1
    slots = nchunks * ni
    nflush = sum(1 for o in ops if o[0] == "flush")
    nstage_rows = nseg * 128
    gidx_all, pos_all = [], []
    for c, (lens, starts, order) in enumerate(core_runs):
        keys, vals = per_core_keys_vals[c]
        vals = np.asarray(vals, np.int64)
        gidx = np.full(slots, dummy_vals[c], np.int64)
        pos = np.zeros(total_keys, np.int64)
        for s in range(nseg):
            ch, blk0 = seg_pos[s]
            k = int(seg_k[s])
            n = min(128, total_keys - s * 128)
            base = ch * ni + blk0 * 128
            ln_seg = lens[s * 128:s * 128 + n]
            st_seg = starts[s * 128:s * 128 + n]
            for j in range(n):
                ln, st = int(ln_seg[j]), int(st_seg[j])
                if ln > 0:
                    gidx[base + j:base + j + ln * 128:128] = vals[st:st + ln]
            pos[order[s * 128:s * 128 + n]] = s * 128 + np.arange(n)
        gidx_all.append(gidx)
        pos_all.append(pos)
    return PassPlan(gidx_all, pos_all, ops, nchunks, slots, nflush, nstage_rows)


def wrap16(flat):
    flat = np.asarray(flat, np.int64)
    assert len(flat) % 16 == 0
    b = flat.reshape(-1, 16).T.astype(np.int16)
    return np.tile(b, (8, 1))


def build_plans(node_idx, edge_idx):
    node_idx = np.asarray(node_idx, np.int64)
    edge_idx = np.asarray(edge_idx, np.int64)
    pa_in, pb_in = [], []
    for m in range(NC):
        sel = np.flatnonzero(node_idx // NSH == m)
        nl = node_idx[sel] - m * NSH
        eg = edge_idx[sel]
        pa_in.append((eg, nl))
        ob = np.argsort(nl, kind="stable")
        pb_in.append((nl[ob], eg[ob]))
    pa = plan_pass(pa_in, ET, [DUMMY_A] * NC)
    pb = plan_pass(pb_in, NSH, [DUMMY_B] * NC)
    return pa, pb


def _fix_chunks(total, ni=NI):
    out = []
    r = 0
    while r < total:
        out.append(min(ni, total - r))
        r += out[-1]
    return out


# ===================== bass builder =====================

def build_bass(pa: PassPlan, pb: PassPlan, replica_groups):
    nc = bacc.Bacc(get_trn_type() or "TRN2", target_bir_lowering=False,
                   debug=False, num_swdge_queues=4)
    qrr = [0]

    def q():
        qrr[0] += 1
        return qrr[0] % 4

    SA = pa.slots // 16
    SB = pb.slots // 16
    FXA = _fix_chunks(ET)
    FXB = _fix_chunks(NT)
    SFA = (ET + 15) // 16        # fixA idx cols (25600/16)
    SFB = (NT + 15) // 16

    # ---- I/O ----
    xT = nc.dram_tensor("xT", [128, NT], BF16, kind="ExternalInput")
    haTm = nc.dram_tensor("haTm", [128, ESH], BF16, kind="ExternalInput")
    Wcomb = nc.dram_tensor("Wcomb", [128, 132], BF16, kind="ExternalInput")
    We_t = nc.dram_tensor("We_t", [128, 128], BF16, kind="ExternalInput")
    bias_t = nc.dram_tensor("bias_t", [128, 128], F32, kind="ExternalInput")
    gidxA = nc.dram_tensor("gidxA", [128, SA], I16, kind="ExternalInput")
    gidxB = nc.dram_tensor("gidxB", [128, SB], I16, kind="ExternalInput")
    fixA_i = nc.dram_tensor("fixA_i", [128, SFA], I16, kind="ExternalInput")
    fixB_i = nc.dram_tensor("fixB_i", [128, SFB], I16, kind="ExternalInput")
    y = nc.dram_tensor("y", [NT, 128], F32, kind="ExternalOutput")

    # ---- internal DRAM ----
    warm_a = nc.dram_tensor("warm_a", [128, 16], BF16)
    warm_b = nc.dram_tensor("warm_b", [128, 16], BF16, addr_space="Shared")
    snf_hbm = nc.dram_tensor("snf_hbm", [NT, 256], BF16)
    Uperm = nc.dram_tensor("Uperm", [pa.nstage_rows, 256], BF16)
    U_hbm = [nc.dram_tensor(f"U_hbm{i}", [ESPL, 256], BF16)
             for i in range(NSPLIT)]
    U_rs = [nc.dram_tensor(f"U_rs{i}", [ESHS, 256], BF16)
            for i in range(NSPLIT)]
    EAp_sh = [nc.dram_tensor(f"EAp_sh{i}", [ESHS, 128], BF16)
              for i in range(NSPLIT)]
    EAp_full = nc.dram_tensor("EAp_full", [ET, 128], BF16, addr_space="Shared")
    EAp_loc = nc.dram_tensor("EAp_loc", [ET, 128], BF16)
    yperm = nc.dram_tensor("yperm", [pb.nstage_rows, 128], BF16)

    with tile.TileContext(nc) as tc, \
         nc.allow_low_precision(reason="bf16 staging validated vs reference"):
        with tc.tile_pool(name="const", bufs=1) as cpool:
            nc.gpsimd.load_library(mlp_library)
            # warm up the collective stack (first collective pays ~145us
            # ENCD/NCCL init); overlaps phase1 since Pool is otherwise idle
            wz = cpool.tile([128, 16], BF16)
            nc.vector.memset(wz[:], 0.0)
            nc.sync.dma_start(warm_a[:], wz[:])
            nc.gpsimd.collective_compute(
                "AllReduce", ALU.add, replica_groups=replica_groups,
                ins=[warm_a[:]], outs=[warm_b[:]])
            Wc_sb = cpool.tile([128, 132], BF16)
            We_sb = cpool.tile([128, 128], BF16)
            bias_sb = cpool.tile([128, 128], F32)
            xsb = cpool.tile([128, NT], BF16)
            giA = cpool.tile([128, SA], I16)
            giB = cpool.tile([128, SB], I16)
            fiA = cpool.tile([128, SFA], I16)
            fiB = cpool.tile([128, SFB], I16)
            nc.sync.dma_start(Wc_sb[:], Wcomb[:])
            nc.sync.dma_start(We_sb[:], We_t[:])
            nc.sync.dma_start(bias_sb[:], bias_t[:])
            nc.sync.dma_start(xsb[:], xT[:])
            nc.sync.dma_start(giA[:], gidxA[:])
            nc.sync.dma_start(giB[:], gidxB[:])
            nc.sync.dma_start(fiA[:], fixA_i[:])
            nc.sync.dma_start(fiB[:], fixB_i[:])

            # ---------- phase 1: tokens ----------
            with (tc.tile_pool(name="p1", bufs=4) as p1,
                  tc.tile_pool(name="ps1", bufs=4, space="PSUM") as ps1):
                for i in range(NT // 128):
                    mm = ps1.tile([128, 132], F32, tag="mm")
                    nc.tensor.matmul(mm[:], xsb[:, i * 128:(i + 1) * 128],
                                     Wc_sb[:], start=True, stop=True)
                    tok = p1.tile([128, 256], BF16, tag="tok")
                    et = p1.tile([128, 4], BF16, tag="et")
                    nc.scalar.activation(et[:], mm[:, 128:132], ACTF.Exp)
                    nc.vector.tensor_tensor(
                        out=tok[:, 0:128].rearrange("p (h c) -> p h c", h=H),
                        in0=mm[:, 0:128].rearrange("p (h c) -> p h c", h=H),
                        in1=et[:].unsqueeze(2).broadcast_to([128, H, C]),
                        op=ALU.mult)
                    nc.vector.tensor_copy(tok[:, 128:132], et[:])
                    nc.scalar.memzero(tok[:, 132:256])
                    nc.sync.dma_start(snf_hbm[i * 128:(i + 1) * 128, :], tok[:])

            # ---------- pass A ----------
            def run_pass(plan, gi, table, elem, red_elem, perm_out, depth, tag):
                ops = plan.ops
                cur_chunk = -1
                gt = None
                stag = None
                spool_ctx = tc.tile_pool(name=f"st{tag}", bufs=3)
                gpool_ctx = tc.tile_pool(name=f"g{tag}", bufs=depth)
                fidx = 0
                # merge consecutive same-k full segments into one TR
                mops = []
                for op in ops:
                    if (op[0] == "tr" and mops and mops[-1][0] == "trm"
                            and mops[-1][1] == op[1]          # same chunk
                            and mops[-1][3] == op[3]          # same k
                            and op[4] == 128
                            and mops[-1][2] + mops[-1][3] * mops[-1][5] == op[2]
                            and mops[-1][6] + mops[-1][5] == op[5]):
                        m = mops[-1]
                        mops[-1] = ("trm", m[1], m[2], m[3], m[4], m[5] + 1,
                                    m[6])
                    elif op[0] == "tr":
                        _, ch, blk0, k, n, sblk = op
                        mops.append(("trm", ch, blk0, k, n, 1, sblk))
                    else:
                        mops.append(op)
                with spool_ctx as spool, gpool_ctx as gpool:
                    for op in mops:
                        if op[0] == "trm":
                            _, ch, blk0, k, n, m, sblk = op
                            if ch != cur_chunk:
                                cur_chunk = ch
                                gt = gpool.tile([128, NBLK * elem], BF16,
                                                tag=f"gt{tag}")
                                nc.gpsimd.dma_gather(
                                    gt[:].rearrange("p (a c) -> p a c", c=elem),
                                    table[:],
                                    gi[:, ch * (NI // 16):(ch + 1) * (NI // 16)],
                                    NI, NI, elem,
                                    single_packet=False, queue_num=q())
                            if stag is None:
                                stag = spool.tile([128, FSEG * elem], BF16,
                                                  tag=f"st{tag}")
                                if red_elem != elem:
                                    nc.scalar.memzero(
                                        stag[:].rearrange(
                                            "p (s c) -> p s c", c=elem)
                                        [:, :, red_elem:elem])
                            base_ap = gt[:]
                            stag_ap = stag[:]
                            iap = bass_rust.AP(
                                base_ap.tensor, base_ap.offset + blk0 * elem,
                                [base_ap.ap[0], [k * elem, m],
                                 [1, red_elem], [elem, k]])
                            oap = bass_rust.AP(
                                stag_ap.tensor,
                                stag_ap.offset + sblk * elem,
                                [stag_ap.ap[0], [elem, m], [1, red_elem]])
                            nc.vector.tensor_reduce(
                                out=oap, in_=iap, axis=AX.X, op=ALU.add)
                        else:
                            _, nseg = op
                            eng = nc.sync if fidx % 2 == 0 else nc.scalar
                            eng.dma_start(
                                perm_out[fidx * FSEG * 128:
                                         fidx * FSEG * 128 + nseg * 128, :]
                                .rearrange("(a p) c -> p a c", p=128),
                                stag[:, 0:nseg * elem]
                                .rearrange("p (a c) -> p a c", c=elem))
                            stag = None
                            fidx += 1

            run_pass(pa, giA, snf_hbm, 256, 132, Uperm, DEPTH_A, "A")

            # ---------- fixA + pipelined RS / EA / AG ----------
            with (tc.tile_pool(name="fxA", bufs=3) as fpool,
                  tc.tile_pool(name="ea", bufs=2) as eap,
                  tc.tile_pool(name="eaps", bufs=4, space="PSUM") as eapp):
                ha_sb = eap.tile([128, ESH], BF16, bufs=1)
                nc.sync.dma_start(ha_sb[:], haTm[:])
                nblkS = ESPL // 128
                for i in range(NSPLIT):
                    ft = fpool.tile([128, nblkS * 256], BF16, tag="ftA",
                                    bufs=NSPLIT)
                    nc.gpsimd.dma_gather(
                        ft[:].rearrange("p (a c) -> p a c", c=256), Uperm[:],
                        fiA[:, i * (ESPL // 16):(i + 1) * (ESPL // 16)],
                        ESPL, ESPL, 256,
                        single_packet=False, queue_num=q())
                    eng = nc.sync if i % 2 == 0 else nc.scalar
                    eng.dma_start(
                        U_hbm[i][:].rearrange("(a p) c -> p a c", p=128),
                        ft[:].rearrange("p (a c) -> p a c", c=256))
                for i in range(NSPLIT):
                    nc.gpsimd.collective_compute(
                        "ReduceScatter", ALU.add, replica_groups=replica_groups,
                        ins=[U_hbm[i][:]], outs=[U_rs[i][:]])
                for i in range(NSPLIT):
                    usb = eap.tile([128, ESHS // 128, 256], BF16, tag="usbE")
                    nc.sync.dma_start(
                        usb[:], U_rs[i][:].rearrange("(a p) c -> p a c", p=128))
                    dv = eap.tile([128, ESHS // 128, 4], F32, tag="dvE")
                    nc.vector.tensor_scalar_add(dv[:], usb[:, :, 128:132], 1e-30)
                    nc.vector.reciprocal(dv[:], dv[:])
                    easb = eap.tile([128, ESHS // 128, 128], BF16, tag="eaE")
                    for a in range(ESHS // 128):
                        mm = eapp.tile([128, 128], F32, tag="mmE")
                        nc.tensor.matmul(
                            mm[:], ha_sb[:, i * ESHS + a * 128:
                                         i * ESHS + (a + 1) * 128],
                            We_sb[:], start=True, stop=True)
                        t1 = eap.tile([128, 128], F32, tag="t1E")
                        nc.vector.tensor_tensor(
                            out=t1[:].rearrange("p (h c) -> p h c", h=H),
                            in0=usb[:, a, 0:128]
                            .rearrange("p (h c) -> p h c", h=H),
                            in1=dv[:, a, :].unsqueeze(2)
                            .broadcast_to([128, H, C]),
                            op=ALU.mult)
                        nc.vector.tensor_tensor(out=t1[:], in0=t1[:],
                                                in1=mm[:], op=ALU.add)
                        nc.vector.tensor_tensor(
                            out=easb[:, a, :].rearrange("p (h c) -> p h c", h=H),
                            in0=t1[:].rearrange("p (h c) -> p h c", h=H),
                            in1=dv[:, a, :].unsqueeze(2)
                            .broadcast_to([128, H, C]),
                            op=ALU.mult)
                    nc.scalar.dma_start(
                        EAp_sh[i][:].rearrange("(a p) c -> p a c", p=128),
                        easb[:])
                    nc.gpsimd.collective_compute(
                        "AllGather", ALU.bypass, replica_groups=replica_groups,
                        ins=[EAp_sh[i][:]],
                        outs=[EAp_full[i * ESPL:(i + 1) * ESPL, :]])
                    nc.sync.dma_start(EAp_loc[i * ESPL:(i + 1) * ESPL, :],
                                      EAp_full[i * ESPL:(i + 1) * ESPL, :])

            # ---------- pass B ----------
            run_pass(pb, giB, EAp_loc, 128, 128, yperm, DEPTH_B, "B")

            # ---------- fixB: unpermute + *exp + bias ----------
            with tc.tile_pool(name="fxB", bufs=3) as fpool:
                r0 = 0
                off = 0
                for nr in FXB:
                    na = nr // 128
                    ft = fpool.tile([128, NBLK * 128], BF16, tag="ftB")
                    nc.gpsimd.dma_gather(
                        ft[:, 0:na * 128]
                        .rearrange("p (a c) -> p a c", c=128), yperm[:],
                        fiB[:, off:off + nr // 16], nr, nr, 128,
                        single_packet=False, queue_num=q())
                    et = fpool.tile([128, NBLK, 4], BF16, tag="etB")
                    nc.scalar.dma_start(
                        et[:, 0:na, :],
                        snf_hbm[r0:r0 + nr, 128:132]
                        .rearrange("(a p) c -> p a c", p=128))
                    yo = fpool.tile([128, NBLK, 128], F32, tag="yoB")
                    nc.vector.tensor_tensor(
                        out=yo[:, 0:na, :].rearrange("p a (h c) -> p a h c", h=H),
                        in0=ft[:, 0:na * 128]
                        .rearrange("p (a h c) -> p a h c", a=na, h=H),
                        in1=et[:, 0:na, :].unsqueeze(3)
                        .broadcast_to([128, na, H, C]),
                        op=ALU.mult)
                    nc.vector.tensor_tensor(
                        out=yo[:, 0:na, :],
                        in0=yo[:, 0:na, :],
                        in1=bias_sb[:].unsqueeze(1).broadcast_to([128, na, 128]),
                        op=ALU.add)
                    nc.sync.dma_start(
                        y[r0:r0 + nr, :].rearrange("(a p) c -> p a c", p=128),
                        yo[:, 0:na, :])
                    r0 += nr
                    off += nr // 16
    nc.compile()
    return nc


# ===================== host glue =====================

def host_inputs(x, ha, W_node, W_edge, attn_l, bias, pa, pb):
    import ml_dtypes
    x = np.asarray(x, np.float64)
    ha = np.asarray(ha, np.float64)
    Wn = np.asarray(W_node, np.float64)
    We = np.asarray(W_edge, np.float64)
    attn = np.asarray(attn_l, np.float64).reshape(H, C)
    bias = np.asarray(bias, np.float32).reshape(-1)
    Wattn = np.zeros((D, H))
    for h in range(H):
        Wattn[:, h] = Wn[:, h * C:(h + 1) * C] @ attn[h]
    v, *_ = np.linalg.lstsq(Wattn.T, np.full(H, -150.0), rcond=None)
    Wcomb = np.concatenate([Wn, Wattn], axis=1).astype(ml_dtypes.bfloat16)
    We_b = We.astype(ml_dtypes.bfloat16)
    bias_rep = np.tile(bias[None, :], (128, 1)).astype(np.float32)
    ha_pad = np.zeros((ET, D), np.float64)
    ha_pad[:E] = ha
    haT_all = ha_pad.T.astype(ml_dtypes.bfloat16)     # [128, ET]
    # fixB idx: nodes 0..NT-1; pad nodes -> pos 0 (garbage, sliced off)
    in_maps = []
    for m in range(NC):
        xm = np.zeros((NT, D), np.float64)
        xm[:NSH] = x[m * NSH:(m + 1) * NSH]
        xm[NSH:] = v[None, :]
        posB = np.zeros(NT, np.int64)
        posB[:NSH] = pb.pos[m][:NSH]
        ham = np.concatenate(
            [haT_all[:, i * ESPL + m * ESHS:i * ESPL + (m + 1) * ESHS]
             for i in range(NSPLIT)], axis=1)
        in_maps.append({
            "xT": np.ascontiguousarray(xm.T).astype(ml_dtypes.bfloat16),
            "haTm": np.ascontiguousarray(ham),
            "Wcomb": Wcomb,
            "We_t": We_b,
            "bias_t": bias_rep,
            "gidxA": wrap16(pa.gidx[m]),
            "gidxB": wrap16(pb.gidx[m]),
            "fixA_i": wrap16(pa.pos[m]),
            "fixB_i": wrap16(posB),
        })
    return in_maps


LAST_RESULTS = None


def _install_axon_ntff_shim():
    import sys, types, ctypes, contextlib
    import concourse.bass_utils as bu
    bu.upload_artifacts = lambda d: str(d)
    try:
        from antenv.axon_hooks import get_axon_ntff_profile_hook  # noqa
        return
    except ImportError:
        pass
    so_path = "/opt/axon/libaxon_pjrt.so"
    try:
        lib = ctypes.CDLL(so_path)
    except OSError:
        return
    if not hasattr(lib, "axon_start_nrt_profile"):
        return
    lib.axon_start_nrt_profile.argtypes = [ctypes.POINTER(ctypes.c_int64),
                                           ctypes.c_size_t]
    lib.axon_start_nrt_profile.restype = ctypes.c_int64
    lib.axon_stop_nrt_profile.argtypes = [ctypes.c_char_p]
    lib.axon_stop_nrt_profile.restype = ctypes.c_int64

    @contextlib.contextmanager
    def _hook(output_dir, device_ids):
        import jax
        jax.devices()
        if device_ids:
            ids = (ctypes.c_int64 * len(device_ids))(*device_ids)
            rc = lib.axon_start_nrt_profile(ids, len(device_ids))
        else:
            rc = lib.axon_start_nrt_profile(None, 0)
        if rc != 0:
            raise RuntimeError(f"axon_start_nrt_profile rc={rc}")
        try:
            yield
        finally:
            n = lib.axon_stop_nrt_profile(str(output_dir).encode())
            print(f"ntff profile: {n} file(s) -> {output_dir}")

    mod = types.ModuleType("antenv.axon_hooks")
    mod.get_axon_ntff_profile_hook = lambda: _hook
    mod.set_axon_ntff_profile_hook = lambda h: None
    sys.modules["antenv.axon_hooks"] = mod


def kernel(**inputs) -> np.ndarray:
    from concourse.bass_utils import run_bass_kernel_spmd
    x = np.asarray(inputs["x"], np.float32)
    ha = np.asarray(inputs["hyperedge_attr"], np.float32)
    node_idx = np.asarray(inputs["node_idx"]).astype(np.int64)
    edge_idx = np.asarray(inputs["edge_idx"]).astype(np.int64)
    pa, pb = build_plans(node_idx, edge_idx)
    nc = build_bass(pa, pb, [list(range(NC))])
    in_maps = host_inputs(x, ha, inputs["W_node"], inputs["W_edge"],
                          inputs["attn_l"], inputs["bias"], pa, pb)
    trace = os.environ.get("GNN_TRACE", "0") == "1"
    if trace:
        _install_axon_ntff_shim()
    res = run_bass_kernel_spmd(nc, in_maps, list(range(NC)), trace=trace)
    global LAST_RESULTS
    LAST_RESULTS = res
    out = np.concatenate(
        [np.asarray(res.results[m]["y"])[:NSH] for m in range(NC)], axis=0)
    return np.ascontiguousarray(out, dtype=np.float32)


# revision 6
# speedup vs baseline: 1.0004x; 1.0004x over previous
"""Token-major hypergraph conv kernel for TRN2 (8 cores, SPMD).

Design:
  - phase1: per 128-node chunk, one bf16 matmul [x^T chunk | Wn|Wattn] ->
    tokens [snf(128) | exp(4) | pad] written to snf_hbm (node-major, 512B rows).
  - pass A: per-incidence non-transpose dma_gather of 512B tokens (runs of a
    segment live on one partition row); free-dim tensor_reduce over k ->
    token-major run sums; staging flushed DENSE to Uperm_hbm (HWDGE).
  - fixA: inverse-permutation gather -> dense U_hbm [25600, 256] (U128|D4).
  - ReduceScatter(U_hbm) -> per-rank [3200, 256]; EA' = (U/D + ef)/D with ef
    matmul'd on the fly; AllGather -> EAp_full [25600, 128] bf16.
  - pass B: same gather/reduce over node-sorted incidences -> yperm_hbm.
  - fixB: inverse-perm gather + *exp + bias -> y [12544, 128] f32.
"""
import os
import numpy as np
from dataclasses import dataclass

import bass_rust
import concourse.bass as bass
import concourse.mybir as mybir
import concourse.bacc as bacc
import concourse.tile as tile
from concourse.library_config import mlp as mlp_library
from concourse._compat import get_trn_type

F32 = mybir.dt.float32
BF16 = mybir.dt.bfloat16
I16 = mybir.dt.int16
AX = mybir.AxisListType
ALU = mybir.AluOpType
ACTF = mybir.ActivationFunctionType

N, E, D, H, C = 100000, 25000, 128, 4, 32
NC, NSH = 8, 12500
ET = 25600              # padded edges (200*128; /8 = 3200 = 25*128)
NT = 12544              # node tokens (98*128; 12500 real + dummy at 12500)
ESH = ET // NC          # 3200 edge rows per rank after RS
NSPLIT = 5              # collective pipeline slices
ESPL = ET // NSPLIT     # 5120 edges per slice
ESHS = ESPL // NC       # 640 rows per rank per slice
DUMMY_A = NSH           # pass A dummy gather target (zero token)
DUMMY_B = ET - 1        # pass B dummy gather target (EAp row == 0)

NI = 4096               # gather idxs per instruction
NBLK = NI // 128
FSEG = 16               # segments per staging flush
DEPTH_A = 8             # gather tile rotation depth
DEPTH_B = 8


# ===================== host planner =====================

def _runs(keys):
    if len(keys) == 0:
        return np.zeros(0, np.int64), np.zeros(0, np.int64), np.zeros(0, np.int64)
    change = np.flatnonzero(np.diff(keys)) + 1
    starts = np.concatenate([[0], change]).astype(np.int64)
    ends = np.concatenate([change, [len(keys)]]).astype(np.int64)
    return starts, ends - starts, keys[starts].astype(np.int64)


@dataclass
class PassPlan:
    gidx: list      # per core [slots] gather token ids
    pos: list       # per core [total_keys] staging row of each key
    ops: list       # common ("tr", chunk, blk0, k, n, sblk) / ("flush", nseg)
    nchunks: int
    slots: int
    nflush: int
    nstage_rows: int


def plan_pass(per_core_keys_vals, total_keys, dummy_vals, ni=NI):
    core_runs = []
    for keys, vals in per_core_keys_vals:
        starts, lens, kv = _runs(np.asarray(keys, np.int64))
        run_len = np.zeros(total_keys, np.int64)
        run_start = np.full(total_keys, -1, np.int64)
        run_len[kv] = lens
        run_start[kv] = starts
        order = np.argsort(-np.maximum(run_len, 1), kind="stable")
        core_runs.append((run_len[order], run_start[order], order))
    nseg = (total_keys + 127) // 128
    seg_k = np.zeros(nseg, np.int64)
    for s in range(nseg):
        for lens, _, _ in core_runs:
            seg_k[s] = max(seg_k[s], max(int(lens[s * 128]), 1))
    nblk = ni // 128
    assert seg_k.max() <= nblk, f"seg width {seg_k.max()} > {nblk}"
    ops = []
    seg_pos = []
    chunk = blk = 0
    fseg = 0
    for s in range(nseg):
        k = int(seg_k[s])
        n = min(128, total_keys - s * 128)
        if blk + k > nblk:
            chunk += 1
            blk = 0
        ops.append(("tr", chunk, blk, k, n, fseg))
        seg_pos.append((chunk, blk))
        blk += k
        fseg += 1
        if fseg == FSEG:
            ops.append(("flush", fseg))
            fseg = 0
    if fseg:
        ops.append(("flush", fseg))
    nchunks = chunk + 1
    slots = nchunks * ni
    nflush = sum(1 for o in ops if o[0] == "flush")
    nstage_rows = nseg * 128
    gidx_all, pos_all = [], []
    for c, (lens, starts, order) in enumerate(core_runs):
        keys, vals = per_core_keys_vals[c]
        vals = np.asarray(vals, np.int64)
        gidx = np.full(slots, dummy_vals[c], np.int64)
        pos = np.zeros(total_keys, np.int64)
        for s in range(nseg):
            ch, blk0 = seg_pos[s]
            k = int(seg_k[s])
            n = min(128, total_keys - s * 128)
            base = ch * ni + blk0 * 128
            ln_seg = lens[s * 128:s * 128 + n]
            st_seg = starts[s * 128:s * 128 + n]
            for j in range(n):
                ln, st = int(ln_seg[j]), int(st_seg[j])
                if ln > 0:
                    gidx[base + j:base + j + ln * 128:128] = vals[st:st + ln]
            pos[order[s * 128:s * 128 + n]] = s * 128 + np.arange(n)
        gidx_all.append(gidx)
        pos_all.append(pos)
    return PassPlan(gidx_all, pos_all, ops, nchunks, slots, nflush, nstage_rows)


def wrap16(flat):
    flat = np.asarray(flat, np.int64)
    assert len(flat) % 16 == 0
    b = flat.reshape(-1, 16).T.astype(np.int16)
    return np.tile(b, (8, 1))


def build_plans(node_idx, edge_idx):
    node_idx = np.asarray(node_idx, np.int64)
    edge_idx = np.asarray(edge_idx, np.int64)
    pa_in, pb_in = [], []
    for m in range(NC):
        sel = np.flatnonzero(node_idx // NSH == m)
        nl = node_idx[sel] - m * NSH
        eg = edge_idx[sel]
        pa_in.append((eg, nl))
        ob = np.argsort(nl, kind="stable")
        pb_in.append((nl[ob], eg[ob]))
    pa = plan_pass(pa_in, ET, [DUMMY_A] * NC)
    pb = plan_pass(pb_in, NSH, [DUMMY_B] * NC)
    return pa, pb


def _fix_chunks(total, ni=NI):
    out = []
    r = 0
    while r < total:
        out.append(min(ni, total - r))
        r += out[-1]
    return out


# ===================== bass builder =====================

def build_bass(pa: PassPlan, pb: PassPlan, replica_groups):
    nc = bacc.Bacc(get_trn_type() or "TRN2", target_bir_lowering=False,
                   debug=False, num_swdge_queues=4)
    qrr = [0]

    def q():
        qrr[0] += 1
        return qrr[0] % 4

    SA = pa.slots // 16
    SB = pb.slots // 16
    FXA = _fix_chunks(ET)
    FXB = _fix_chunks(NT)
    SFA = (ET + 15) // 16        # fixA idx cols (25600/16)
    SFB = (NT + 15) // 16

    # ---- I/O ----
    xT = nc.dram_tensor("xT", [128, NT], BF16, kind="ExternalInput")
    haTm = nc.dram_tensor("haTm", [128, ESH], BF16, kind="ExternalInput")
    Wcomb = nc.dram_tensor("Wcomb", [128, 132], BF16, kind="ExternalInput")
    We_t = nc.dram_tensor("We_t", [128, 128], BF16, kind="ExternalInput")
    bias_t = nc.dram_tensor("bias_t", [128, 128], F32, kind="ExternalInput")
    gidxA = nc.dram_tensor("gidxA", [128, SA], I16, kind="ExternalInput")
    gidxB = nc.dram_tensor("gidxB", [128, SB], I16, kind="ExternalInput")
    fixA_i = nc.dram_tensor("fixA_i", [128, SFA], I16, kind="ExternalInput")
    fixB_i = nc.dram_tensor("fixB_i", [128, SFB], I16, kind="ExternalInput")
    y = nc.dram_tensor("y", [NT, 128], F32, kind="ExternalOutput")

    # ---- internal DRAM ----
    warm_a = nc.dram_tensor("warm_a", [128, 16], BF16)
    warm_b = nc.dram_tensor("warm_b", [128, 16], BF16, addr_space="Shared")
    snf_hbm = nc.dram_tensor("snf_hbm", [NT, 256], BF16)
    Uperm = nc.dram_tensor("Uperm", [pa.nstage_rows, 256], BF16)
    U_hbm = [nc.dram_tensor(f"U_hbm{i}", [ESPL, 256], BF16)
             for i in range(NSPLIT)]
    U_rs = [nc.dram_tensor(f"U_rs{i}", [ESHS, 256], BF16)
            for i in range(NSPLIT)]
    EAp_sh = [nc.dram_tensor(f"EAp_sh{i}", [ESHS, 128], BF16)
              for i in range(NSPLIT)]
    EAp_full = nc.dram_tensor("EAp_full", [ET, 128], BF16, addr_space="Shared")
    EAp_loc = nc.dram_tensor("EAp_loc", [ET, 128], BF16)
    yperm = nc.dram_tensor("yperm", [pb.nstage_rows, 128], BF16)

    with tile.TileContext(nc) as tc, \
         nc.allow_low_precision(reason="bf16 staging validated vs reference"):
        with tc.tile_pool(name="const", bufs=1) as cpool:
            nc.gpsimd.load_library(mlp_library)
            # warm up the collective stack (first collective pays ~145us
            # ENCD/NCCL init); overlaps phase1 since Pool is otherwise idle
            wz = cpool.tile([128, 16], BF16)
            nc.vector.memset(wz[:], 0.0)
            nc.sync.dma_start(warm_a[:], wz[:])
            nc.gpsimd.collective_compute(
                "AllReduce", ALU.add, replica_groups=replica_groups,
                ins=[warm_a[:]], outs=[warm_b[:]])
            Wc_sb = cpool.tile([128, 132], BF16)
            We_sb = cpool.tile([128, 128], BF16)
            bias_sb = cpool.tile([128, 128], F32)
            giA = cpool.tile([128, SA], I16)
            giB = cpool.tile([128, SB], I16)
            fiA = cpool.tile([128, SFA], I16)
            fiB = cpool.tile([128, SFB], I16)
            nc.sync.dma_start(Wc_sb[:], Wcomb[:])
            nc.sync.dma_start(We_sb[:], We_t[:])
            nc.sync.dma_start(bias_sb[:], bias_t[:])
            nc.sync.dma_start(giA[:], gidxA[:])
            nc.sync.dma_start(giB[:], gidxB[:])
            nc.sync.dma_start(fiA[:], fixA_i[:])
            nc.sync.dma_start(fiB[:], fixB_i[:])

            # ---------- phase 1: tokens ----------
            with (tc.tile_pool(name="p1", bufs=4) as p1,
                  tc.tile_pool(name="ps1", bufs=4, space="PSUM") as ps1):
                xsb = p1.tile([128, NT], BF16, bufs=1)
                nc.sync.dma_start(xsb[:], xT[:])
                for i in range(NT // 128):
                    mm = ps1.tile([128, 132], F32, tag="mm")
                    nc.tensor.matmul(mm[:], xsb[:, i * 128:(i + 1) * 128],
                                     Wc_sb[:], start=True, stop=True)
                    tok = p1.tile([128, 132], BF16, tag="tok")
                    nc.scalar.activation(tok[:, 128:132], mm[:, 128:132],
                                         ACTF.Exp)
                    nc.vector.tensor_tensor(
                        out=tok[:, 0:128].rearrange("p (h c) -> p h c", h=H),
                        in0=mm[:, 0:128].rearrange("p (h c) -> p h c", h=H),
                        in1=tok[:, 128:132].unsqueeze(2)
                        .broadcast_to([128, H, C]),
                        op=ALU.mult)
                    nc.sync.dma_start(
                        snf_hbm[i * 128:(i + 1) * 128, 0:132], tok[:])

            # ---------- pass A ----------
            def run_pass(plan, gi, table, elem, red_elem, perm_out, depth, tag):
                ops = plan.ops
                cur_chunk = -1
                gt = None
                stag = None
                spool_ctx = tc.tile_pool(name=f"st{tag}", bufs=3)
                gpool_ctx = tc.tile_pool(name=f"g{tag}", bufs=depth)
                fidx = 0
                # merge consecutive same-k full segments into one TR
                mops = []
                for op in ops:
                    if (op[0] == "tr" and mops and mops[-1][0] == "trm"
                            and mops[-1][1] == op[1]          # same chunk
                            and mops[-1][3] == op[3]          # same k
                            and op[4] == 128
                            and mops[-1][2] + mops[-1][3] * mops[-1][5] == op[2]
                            and mops[-1][6] + mops[-1][5] == op[5]):
                        m = mops[-1]
                        mops[-1] = ("trm", m[1], m[2], m[3], m[4], m[5] + 1,
                                    m[6])
                    elif op[0] == "tr":
                        _, ch, blk0, k, n, sblk = op
                        mops.append(("trm", ch, blk0, k, n, 1, sblk))
                    else:
                        mops.append(op)
                with spool_ctx as spool, gpool_ctx as gpool:
                    for op in mops:
                        if op[0] == "trm":
                            _, ch, blk0, k, n, m, sblk = op
                            if ch != cur_chunk:
                                cur_chunk = ch
                                gt = gpool.tile([128, NBLK * elem], BF16,
                                                tag=f"gt{tag}")
                                nc.gpsimd.dma_gather(
                                    gt[:].rearrange("p (a c) -> p a c", c=elem),
                                    table[:],
                                    gi[:, ch * (NI // 16):(ch + 1) * (NI // 16)],
                                    NI, NI, elem,
                                    single_packet=False, queue_num=q())
                            if stag is None:
                                stag = spool.tile([128, FSEG * elem], BF16,
                                                  tag=f"st{tag}")
                                if red_elem != elem:
                                    nc.scalar.memzero(
                                        stag[:].rearrange(
                                            "p (s c) -> p s c", c=elem)
                                        [:, :, red_elem:elem])
                            base_ap = gt[:]
                            stag_ap = stag[:]
                            iap = bass_rust.AP(
                                base_ap.tensor, base_ap.offset + blk0 * elem,
                                [base_ap.ap[0], [k * elem, m],
                                 [1, red_elem], [elem, k]])
                            oap = bass_rust.AP(
                                stag_ap.tensor,
                                stag_ap.offset + sblk * elem,
                                [stag_ap.ap[0], [elem, m], [1, red_elem]])
                            nc.vector.tensor_reduce(
                                out=oap, in_=iap, axis=AX.X, op=ALU.add)
                        else:
                            _, nseg = op
                            eng = nc.sync if fidx % 2 == 0 else nc.scalar
                            eng.dma_start(
                                perm_out[fidx * FSEG * 128:
                                         fidx * FSEG * 128 + nseg * 128, :]
                                .rearrange("(a p) c -> p a c", p=128),
                                stag[:, 0:nseg * elem]
                                .rearrange("p (a c) -> p a c", c=elem))
                            stag = None
                            fidx += 1

            run_pass(pa, giA, snf_hbm, 256, 132, Uperm, DEPTH_A, "A")

            # ---------- fixA + pipelined RS / EA / AG ----------
            with (tc.tile_pool(name="fxA", bufs=3) as fpool,
                  tc.tile_pool(name="ea", bufs=2) as eap,
                  tc.tile_pool(name="eaps", bufs=4, space="PSUM") as eapp):
                ha_sb = eap.tile([128, ESH], BF16, bufs=1)
                nc.sync.dma_start(ha_sb[:], haTm[:])
                nblkS = ESPL // 128
                for i in range(NSPLIT):
                    ft = fpool.tile([128, nblkS * 256], BF16, tag="ftA",
                                    bufs=NSPLIT)
                    nc.gpsimd.dma_gather(
                        ft[:].rearrange("p (a c) -> p a c", c=256), Uperm[:],
                        fiA[:, i * (ESPL // 16):(i + 1) * (ESPL // 16)],
                        ESPL, ESPL, 256,
                        single_packet=False, queue_num=q())
                    eng = nc.sync if i % 2 == 0 else nc.scalar
                    eng.dma_start(
                        U_hbm[i][:].rearrange("(a p) c -> p a c", p=128),
                        ft[:].rearrange("p (a c) -> p a c", c=256))
                for i in range(NSPLIT):
                    nc.gpsimd.collective_compute(
                        "ReduceScatter", ALU.add, replica_groups=replica_groups,
                        ins=[U_hbm[i][:]], outs=[U_rs[i][:]])
                for i in range(NSPLIT):
                    usb = eap.tile([128, ESHS // 128, 256], BF16, tag="usbE")
                    nc.sync.dma_start(
                        usb[:], U_rs[i][:].rearrange("(a p) c -> p a c", p=128))
                    dv = eap.tile([128, ESHS // 128, 4], F32, tag="dvE")
                    nc.vector.tensor_scalar_add(dv[:], usb[:, :, 128:132], 1e-30)
                    nc.vector.reciprocal(dv[:], dv[:])
                    easb = eap.tile([128, ESHS // 128, 128], BF16, tag="eaE")
                    for a in range(ESHS // 128):
                        mm = eapp.tile([128, 128], F32, tag="mmE")
                        nc.tensor.matmul(
                            mm[:], ha_sb[:, i * ESHS + a * 128:
                                         i * ESHS + (a + 1) * 128],
                            We_sb[:], start=True, stop=True)
                        t1 = eap.tile([128, 128], F32, tag="t1E")
                        nc.vector.tensor_tensor(
                            out=t1[:].rearrange("p (h c) -> p h c", h=H),
                            in0=usb[:, a, 0:128]
                            .rearrange("p (h c) -> p h c", h=H),
                            in1=dv[:, a, :].unsqueeze(2)
                            .broadcast_to([128, H, C]),
                            op=ALU.mult)
                        nc.vector.tensor_tensor(out=t1[:], in0=t1[:],
                                                in1=mm[:], op=ALU.add)
                        nc.vector.tensor_tensor(
                            out=easb[:, a, :].rearrange("p (h c) -> p h c", h=H),
                            in0=t1[:].rearrange("p (h c) -> p h c", h=H),
                            in1=dv[:, a, :].unsqueeze(2)
                            .broadcast_to([128, H, C]),
                            op=ALU.mult)
                    nc.scalar.dma_start(
                        EAp_sh[i][:].rearrange("(a p) c -> p a c", p=128),
                        easb[:])
                    nc.gpsimd.collective_compute(
                        "AllGather", ALU.bypass, replica_groups=replica_groups,
                        ins=[EAp_sh[i][:]],
                        outs=[EAp_full[i * ESPL:(i + 1) * ESPL, :]])
                    nc.sync.dma_start(EAp_loc[i * ESPL:(i + 1) * ESPL, :],
                                      EAp_full[i * ESPL:(i + 1) * ESPL, :])

            # ---------- pass B ----------
            run_pass(pb, giB, EAp_loc, 128, 128, yperm, DEPTH_B, "B")

            # ---------- fixB: unpermute + *exp + bias ----------
            with tc.tile_pool(name="fxB", bufs=3) as fpool:
                r0 = 0
                off = 0
                for nr in FXB:
                    na = nr // 128
                    ft = fpool.tile([128, NBLK * 128], BF16, tag="ftB")
                    nc.gpsimd.dma_gather(
                        ft[:, 0:na * 128]
                        .rearrange("p (a c) -> p a c", c=128), yperm[:],
                        fiB[:, off:off + nr // 16], nr, nr, 128,
                        single_packet=False, queue_num=q())
                    et = fpool.tile([128, NBLK, 4], BF16, tag="etB")
                    nc.scalar.dma_start(
                        et[:, 0:na, :],
                        snf_hbm[r0:r0 + nr, 128:132]
                        .rearrange("(a p) c -> p a c", p=128))
                    yo = fpool.tile([128, NBLK, 128], F32, tag="yoB")
                    nc.vector.tensor_tensor(
                        out=yo[:, 0:na, :].rearrange("p a (h c) -> p a h c", h=H),
                        in0=ft[:, 0:na * 128]
                        .rearrange("p (a h c) -> p a h c", a=na, h=H),
                        in1=et[:, 0:na, :].unsqueeze(3)
                        .broadcast_to([128, na, H, C]),
                        op=ALU.mult)
                    nc.vector.tensor_tensor(
                        out=yo[:, 0:na, :],
                        in0=yo[:, 0:na, :],
                        in1=bias_sb[:].unsqueeze(1).broadcast_to([128, na, 128]),
                        op=ALU.add)
                    nc.sync.dma_start(
                        y[r0:r0 + nr, :].rearrange("(a p) c -> p a c", p=128),
                        yo[:, 0:na, :])
                    r0 += nr
                    off += nr // 16
    nc.compile()
    return nc


# ===================== host glue =====================

def host_inputs(x, ha, W_node, W_edge, attn_l, bias, pa, pb):
    import ml_dtypes
    x = np.asarray(x, np.float64)
    ha = np.asarray(ha, np.float64)
    Wn = np.asarray(W_node, np.float64)
    We = np.asarray(W_edge, np.float64)
    attn = np.asarray(attn_l, np.float64).reshape(H, C)
    bias = np.asarray(bias, np.float32).reshape(-1)
    Wattn = np.zeros((D, H))
    for h in range(H):
        Wattn[:, h] = Wn[:, h * C:(h + 1) * C] @ attn[h]
    v, *_ = np.linalg.lstsq(Wattn.T, np.full(H, -150.0), rcond=None)
    Wcomb = np.concatenate([Wn, Wattn], axis=1).astype(ml_dtypes.bfloat16)
    We_b = We.astype(ml_dtypes.bfloat16)
    bias_rep = np.tile(bias[None, :], (128, 1)).astype(np.float32)
    ha_pad = np.zeros((ET, D), np.float64)
    ha_pad[:E] = ha
    haT_all = ha_pad.T.astype(ml_dtypes.bfloat16)     # [128, ET]
    # fixB idx: nodes 0..NT-1; pad nodes -> pos 0 (garbage, sliced off)
    in_maps = []
    for m in range(NC):
        xm = np.zeros((NT, D), np.float64)
        xm[:NSH] = x[m * NSH:(m + 1) * NSH]
        xm[NSH:] = v[None, :]
        posB = np.zeros(NT, np.int64)
        posB[:NSH] = pb.pos[m][:NSH]
        ham = np.concatenate(
            [haT_all[:, i * ESPL + m * ESHS:i * ESPL + (m + 1) * ESHS]
             for i in range(NSPLIT)], axis=1)
        in_maps.append({
            "xT": np.ascontiguousarray(xm.T).astype(ml_dtypes.bfloat16),
            "haTm": np.ascontiguousarray(ham),
            "Wcomb": Wcomb,
            "We_t": We_b,
            "bias_t": bias_rep,
            "gidxA": wrap16(pa.gidx[m]),
            "gidxB": wrap16(pb.gidx[m]),
            "fixA_i": wrap16(pa.pos[m]),
            "fixB_i": wrap16(posB),
        })
    return in_maps


LAST_RESULTS = None


def _install_axon_ntff_shim():
    import sys, types, ctypes, contextlib
    import concourse.bass_utils as bu
    bu.upload_artifacts = lambda d: str(d)
    try:
        from antenv.axon_hooks import get_axon_ntff_profile_hook  # noqa
        return
    except ImportError:
        pass
    so_path = "/opt/axon/libaxon_pjrt.so"
    try:
        lib = ctypes.CDLL(so_path)
    except OSError:
        return
    if not hasattr(lib, "axon_start_nrt_profile"):
        return
    lib.axon_start_nrt_profile.argtypes = [ctypes.POINTER(ctypes.c_int64),
                                           ctypes.c_size_t]
    lib.axon_start_nrt_profile.restype = ctypes.c_int64
    lib.axon_stop_nrt_profile.argtypes = [ctypes.c_char_p]
    lib.axon_stop_nrt_profile.restype = ctypes.c_int64

    @contextlib.contextmanager
    def _hook(output_dir, device_ids):
        import jax
        jax.devices()
        if device_ids:
            ids = (ctypes.c_int64 * len(device_ids))(*device_ids)
            rc = lib.axon_start_nrt_profile(ids, len(device_ids))
        else:
            rc = lib.axon_start_nrt_profile(None, 0)
        if rc != 0:
            raise RuntimeError(f"axon_start_nrt_profile rc={rc}")
        try:
            yield
        finally:
            n = lib.axon_stop_nrt_profile(str(output_dir).encode())
            print(f"ntff profile: {n} file(s) -> {output_dir}")

    mod = types.ModuleType("antenv.axon_hooks")
    mod.get_axon_ntff_profile_hook = lambda: _hook
    mod.set_axon_ntff_profile_hook = lambda h: None
    sys.modules["antenv.axon_hooks"] = mod


def kernel(**inputs) -> np.ndarray:
    from concourse.bass_utils import run_bass_kernel_spmd
    x = np.asarray(inputs["x"], np.float32)
    ha = np.asarray(inputs["hyperedge_attr"], np.float32)
    node_idx = np.asarray(inputs["node_idx"]).astype(np.int64)
    edge_idx = np.asarray(inputs["edge_idx"]).astype(np.int64)
    pa, pb = build_plans(node_idx, edge_idx)
    nc = build_bass(pa, pb, [list(range(NC))])
    in_maps = host_inputs(x, ha, inputs["W_node"], inputs["W_edge"],
                          inputs["attn_l"], inputs["bias"], pa, pb)
    trace = os.environ.get("GNN_TRACE", "0") == "1"
    if trace:
        _install_axon_ntff_shim()
    res = run_bass_kernel_spmd(nc, in_maps, list(range(NC)), trace=trace)
    global LAST_RESULTS
    LAST_RESULTS = res
    out = np.concatenate(
        [np.asarray(res.results[m]["y"])[:NSH] for m in range(NC)], axis=0)
    return np.ascontiguousarray(out, dtype=np.float32)
